# revision 7
# baseline (speedup 1.0000x reference)
"""Trainium2 Bass kernel for nn_DiffusionStar (retrieval_knn).

Computes eps_star = (x - sqrt(ab) * weighted_x) / sqrt(1 - ab) where
weighted_x is the softmax-weighted average of the train set under the
Gaussian kernel exp(-||x - sqrt(ab) x0||^2 / (2 (1 - ab))).

Strategy (per sharding hint): shard train_data along N across 8 cores.
Each core streams its shard once, computing per-tile logits via a PE
matmul (contraction over D=3072 needs a D-major operand, so the host
uploads the shard transposed, split into fp16-hi + scaled-fp8-lo so the
logit precision matches f32), tracks a running row max, exponentiates
with the running max as bias (online softmax), and accumulates the
weighted sum W = p @ train via a second PE matmul off the natively-laid
fp16 copy, rescaling W by exp(m_old - m_new) each tile. Cores return
partial (W, m, s); the host merges with the standard online-softmax
combine and applies the final elementwise formula.
"""

import contextlib

import ml_dtypes
import numpy as np

from concourse import bacc, bass, mybir, tile
from concourse import bass_utils

FP16 = mybir.dt.float16
FP8 = mybir.dt.float8e4
F32 = mybir.dt.float32
NP_FP8 = ml_dtypes.float8_e4m3

B = 32          # queries
D = 3072        # feature dim (c*h*w)
N = 100000      # train points
N_CORES = 8
N_SHARD = N // N_CORES          # 12500
TILE = 512
N_TILES = (N_SHARD + TILE - 1) // TILE   # 25
N_PAD = N_TILES * TILE                   # 12800
KC = D // 128                            # 24 contraction chunks
LO_SCALE = 64.0                          # fp8 lo-residual scale
PAD_BIAS = -30000.0                      # logit bias for padded rows


def build_nc(n_tiles=N_TILES, repeat=1):
    nc = bacc.Bacc("TRN2", target_bir_lowering=False, debug=False, num_devices=1)

    a_hi = nc.dram_tensor("a_hi", [n_tiles, D, TILE], FP16, kind="ExternalInput").ap()
    a_lo = nc.dram_tensor("a_lo", [n_tiles, D, TILE], FP8, kind="ExternalInput").ap()
    a_b = nc.dram_tensor("a_b", [n_tiles, 2, TILE], FP16, kind="ExternalInput").ap()
    b_hi = nc.dram_tensor(
        "b_hi", [n_tiles, 4, 128, D], FP16, kind="ExternalInput"
    ).ap()
    xw16 = nc.dram_tensor("xw16", [KC, 128, 64], FP16, kind="ExternalInput").ap()
    xw8 = nc.dram_tensor("xw8", [KC, 128, 64], FP8, kind="ExternalInput").ap()
    xwb = nc.dram_tensor("xwb", [2, 64], FP16, kind="ExternalInput").ap()
    ident = nc.dram_tensor("ident", [32, 32], F32, kind="ExternalInput").ap()

    w_out = nc.dram_tensor("w_out", [B, D], F32, kind="ExternalOutput").ap()
    m_out = nc.dram_tensor("m_out", [B, 1], F32, kind="ExternalOutput").ap()
    s_out = nc.dram_tensor("s_out", [B, 1], F32, kind="ExternalOutput").ap()

    with tile.TileContext(nc) as tc, contextlib.ExitStack() as st:
        const = st.enter_context(tc.tile_pool(name="const", bufs=1))
        apool = st.enter_context(tc.tile_pool(name="apool", bufs=2))
        bpool = st.enter_context(tc.tile_pool(name="bpool", bufs=2))
        small = st.enter_context(tc.tile_pool(name="small", bufs=3))
        pwork = st.enter_context(tc.tile_pool(name="pwork", bufs=2))
        ps_cross = st.enter_context(tc.tile_pool(name="ps_cross", bufs=1, space="PSUM"))
        ps_pt = st.enter_context(tc.tile_pool(name="ps_pt", bufs=1, space="PSUM"))
        ps_w = st.enter_context(tc.tile_pool(name="ps_w", bufs=1, space="PSUM"))

        xw16_s = const.tile([128, KC, 64], FP16)
        nc.sync.dma_start(xw16_s[:], xw16.rearrange("k p j -> p k j"))
        xw8_s = const.tile([128, KC, 64], FP8)
        nc.sync.dma_start(xw8_s[:], xw8.rearrange("k p j -> p k j"))
        xwb_s = const.tile([2, 64], FP16)
        nc.sync.dma_start(xwb_s[:], xwb)
        ident_s = const.tile([32, 32], F32)
        nc.sync.dma_start(ident_s[:], ident)

        W_acc = const.tile([B, D], F32)
        nc.vector.memset(W_acc[:], 0.0)
        m_run = const.tile([B, 1], F32)
        nc.vector.memset(m_run[:], -1e30)
        s_run = const.tile([B, 1], F32)
        nc.vector.memset(s_run[:], 0.0)

        for i in [t for _ in range(repeat) for t in range(n_tiles)]:
            a16_t = apool.tile([128, KC, TILE], FP16, tag="a16")
            nc.sync.dma_start(a16_t[:], a_hi[i].rearrange("(k p) n -> p k n", p=128))
            a8_t = apool.tile([128, KC, TILE], FP8, tag="a8")
            nc.sync.dma_start(a8_t[:], a_lo[i].rearrange("(k p) n -> p k n", p=128))
            ab_t = apool.tile([2, TILE], FP16, tag="ab")
            nc.sync.dma_start(ab_t[:], a_b[i])
            b16_t = bpool.tile([128, 4, D], FP16, tag="b16")
            nc.sync.dma_start(b16_t[:], b_hi[i].rearrange("c p d -> p c d"))

            cross = ps_cross.tile([64, TILE], F32, tag="cross")
            for k in range(KC):
                nc.tensor.matmul(
                    cross[:], xw16_s[:, k], a16_t[:, k], start=(k == 0), stop=False
                )
            nc.tensor.matmul(cross[:], xwb_s[:], ab_t[:], start=False, stop=False)
            for k in range(KC):
                nc.tensor.matmul(
                    cross[:], xw8_s[:, k], a8_t[:, k],
                    start=False, stop=(k == KC - 1),
                )

            # logits = cross[x_hi rows] + cross[x_lo rows]; one PSUM read per op
            crossb = pwork.tile([B, TILE], F32, tag="crossb")
            nc.scalar.copy(crossb[:], cross[B:64, :])
            logits = pwork.tile([B, TILE], F32, tag="logits")
            nc.vector.tensor_add(logits[:], cross[0:B, :], crossb[:])

            mt = small.tile([B, 1], F32, tag="mt")
            nc.vector.reduce_max(mt[:], logits[:], axis=mybir.AxisListType.X)
            mnew = small.tile([B, 1], F32, tag="mnew")
            nc.vector.tensor_max(mnew[:], mt[:], m_run[:])
            dm = small.tile([B, 1], F32, tag="dm")
            nc.vector.tensor_sub(dm[:], m_run[:], mnew[:])
            fsc = small.tile([B, 1], F32, tag="fsc")
            nc.scalar.activation(fsc[:], dm[:], mybir.ActivationFunctionType.Exp)
            nc.vector.tensor_copy(m_run[:], mnew[:])
            negm = small.tile([B, 1], F32, tag="negm")
            nc.vector.tensor_scalar_mul(negm[:], mnew[:], -1.0)

            p = pwork.tile([B, TILE], F32, tag="p")
            rowsum = small.tile([B, 1], F32, tag="rowsum")
            nc.scalar.activation(
                p[:], logits[:], mybir.ActivationFunctionType.Exp,
                bias=negm[:], scale=1.0, accum_out=rowsum[:],
            )
            nc.vector.scalar_tensor_tensor(
                s_run[:], s_run[:], fsc[:], rowsum[:],
                mybir.AluOpType.mult, mybir.AluOpType.add,
            )

            pTp = ps_pt.tile([128, 128], F32, tag="pT")
            for cb in range(4):
                nc.tensor.transpose(
                    pTp[:, cb * 32 : (cb + 1) * 32],
                    p[:, cb * 128 : (cb + 1) * 128],
                    ident_s[:],
                )
            pT16 = pwork.tile([128, 128], FP16, tag="pT16")
            nc.vector.tensor_copy(pT16[:], pTp[:])

            wp = ps_w.tile([B, D], F32, tag="wp")
            for cb in range(4):
                for jb in range(D // 512):
                    sl = slice(jb * 512, (jb + 1) * 512)
                    nc.tensor.matmul(
                        wp[:, sl],
                        pT16[:, cb * 32 : (cb + 1) * 32],
                        b16_t[:, cb, sl],
                        start=(cb == 0),
                        stop=(cb == 3),
                    )
            nc.vector.scalar_tensor_tensor(
                W_acc[:], W_acc[:], fsc[:], wp[:],
                mybir.AluOpType.mult, mybir.AluOpType.add,
            )

        nc.sync.dma_start(w_out, W_acc[:])
        nc.sync.dma_start(m_out, m_run[:])
        nc.sync.dma_start(s_out, s_run[:])

    nc.compile()
    return nc


_NC_CACHE = {}


def _get_nc(n_tiles=N_TILES):
    if n_tiles not in _NC_CACHE:
        _NC_CACHE[n_tiles] = build_nc(n_tiles)
    return _NC_CACHE[n_tiles]


LAST_RESULT = None  # BassKernelResults of the most recent run (for test harness)
LAST_IN_MAPS = None  # per-core input dicts of the most recent run


def kernel(x, train_data, alphas_cumprod, t):
    x = np.asarray(x)
    train_data = np.asarray(train_data)
    alphas_cumprod = np.asarray(alphas_cumprod)
    t_idx = int(np.asarray(t))

    ab = float(alphas_cumprod[t_idx])
    s_ab = np.sqrt(ab)
    one_minus = 1.0 - ab
    coefA = s_ab / one_minus            # logits = coefA * (x . t) - coefB * t_sq
    coefB = ab / (2.0 * one_minus)
    inv = 1.0 / np.sqrt(one_minus)

    xf = x.reshape(B, D).astype(np.float64)
    xs = coefA * xf                      # fold coefA into the query side

    # x-side stationary operands (shared across cores)
    xs_hi = xs.astype(np.float16)
    xs_lo = (xs - xs_hi.astype(np.float64)).astype(np.float16)
    x8 = xs / LO_SCALE
    x8_hi = x8.astype(NP_FP8)
    x8_lo = (x8 - x8_hi.astype(np.float64)).astype(NP_FP8)
    xw16 = np.zeros((KC, 128, 64), np.float16)
    xw8 = np.zeros((KC, 128, 64), NP_FP8)
    for k in range(KC):
        sl = slice(k * 128, (k + 1) * 128)
        xw16[k, :, 0:B] = xs_hi[:, sl].T
        xw16[k, :, B:64] = xs_lo[:, sl].T
        xw8[k, :, 0:B] = x8_hi[:, sl].T
        xw8[k, :, B:64] = x8_lo[:, sl].T
    xwb = np.zeros((2, 64), np.float16)
    xwb[0, 0:B] = 1.0
    xwb[1, 0:B] = 1.0
    ident = np.eye(32, dtype=np.float32)

    tf = train_data.reshape(N, D)
    in_maps = []
    for c in range(N_CORES):
        shard = tf[c * N_SHARD : (c + 1) * N_SHARD].astype(np.float32)
        t_pad = np.zeros((N_PAD, D), np.float32)
        t_pad[:N_SHARD] = shard

        td = t_pad.astype(np.float64)
        t_sq = np.einsum("nd,nd->n", td, td)
        bias = -coefB * (t_sq - float(D))
        bias[N_SHARD:] = PAD_BIAS

        At = td.T                                    # [D, N_PAD]
        A_hi16 = At.astype(np.float16)
        A_lo8 = ((At - A_hi16.astype(np.float64)) * LO_SCALE).astype(NP_FP8)
        a_hi = np.ascontiguousarray(
            A_hi16.reshape(D, N_TILES, TILE).transpose(1, 0, 2)
        )
        a_lo = np.ascontiguousarray(
            A_lo8.reshape(D, N_TILES, TILE).transpose(1, 0, 2)
        )

        bias_hi = bias.astype(np.float16)
        bias_lo = (bias - bias_hi.astype(np.float64)).astype(np.float16)
        a_b = np.ascontiguousarray(
            np.stack([bias_hi, bias_lo])             # [2, N_PAD]
            .reshape(2, N_TILES, TILE)
            .transpose(1, 0, 2)
        ).astype(np.float16)

        b_hi = t_pad.astype(np.float16).reshape(N_TILES, 4, 128, D)

        in_maps.append(
            dict(
                a_hi=a_hi, a_lo=a_lo, a_b=a_b, b_hi=b_hi,
                xw16=xw16, xw8=xw8, xwb=xwb, ident=ident,
            )
        )

    nc = _get_nc()
    res = bass_utils.run_bass_kernel_spmd(nc, in_maps, core_ids=list(range(N_CORES)))
    global LAST_RESULT, LAST_IN_MAPS
    LAST_RESULT = res
    LAST_IN_MAPS = in_maps

    Wc = np.stack([r["w_out"] for r in res.results]).astype(np.float64)  # [8,B,D]
    mc = np.stack([r["m_out"][:, 0] for r in res.results]).astype(np.float64)
    sc = np.stack([r["s_out"][:, 0] for r in res.results]).astype(np.float64)

    M = mc.max(0)                                    # [B]
    fac = np.exp(mc - M[None, :])                    # [8, B]
    W_tot = np.einsum("cb,cbd->bd", fac, Wc)
    s_tot = (fac * sc).sum(0)                        # [B]
    weighted = W_tot / s_tot[:, None]                # [B, D]

    out = inv * xf - (s_ab * inv) * weighted
    return out.reshape(x.shape).astype(np.float32)


# revision 10
# speedup vs baseline: 1.1768x; 1.1768x over previous
"""Trainium2 Bass kernel for nn_DiffusionStar (retrieval_knn).

Computes eps_star = (x - sqrt(ab) * weighted_x) / sqrt(1 - ab) where
weighted_x is the softmax-weighted average of the train set under the
Gaussian kernel exp(-||x - sqrt(ab) x0||^2 / (2 (1 - ab))).

Strategy (per sharding hint): shard train_data along N across 8 cores.
Each core streams its shard once, computing per-tile logits via a PE
matmul (contraction over D=3072 needs a D-major operand, so the host
uploads the shard transposed, split into fp16-hi + scaled-fp8-lo so the
logit precision matches f32), tracks a running row max, exponentiates
with the running max as bias (online softmax), and accumulates the
weighted sum W = p @ train via a second PE matmul off the natively-laid
fp16 copy, rescaling W by exp(m_old - m_new) each tile. Cores return
partial (W, m, s); the host merges with the standard online-softmax
combine and applies the final elementwise formula.
"""

import contextlib

import ml_dtypes
import numpy as np

from concourse import bacc, bass, mybir, tile
from concourse import bass_utils

FP16 = mybir.dt.float16
FP8 = mybir.dt.float8e4
F32 = mybir.dt.float32
NP_FP8 = ml_dtypes.float8_e4m3

B = 32          # queries
D = 3072        # feature dim (c*h*w)
N = 100000      # train points
N_CORES = 8
N_SHARD = N // N_CORES          # 12500
TILE = 512
N_TILES = (N_SHARD + TILE - 1) // TILE   # 25
N_PAD = N_TILES * TILE                   # 12800
KC = D // 128                            # 24 contraction chunks
LO_SCALE = 64.0                          # fp8 lo-residual scale
PAD_BIAS = -30000.0                      # logit bias for padded rows


def build_nc(n_tiles=N_TILES, repeat=1, skip_compute=False, skip_dma=False):
    nc = bacc.Bacc("TRN2", target_bir_lowering=False, debug=False, num_devices=1)

    a_hi = nc.dram_tensor("a_hi", [n_tiles, D, TILE], FP16, kind="ExternalInput").ap()
    a_lo = nc.dram_tensor("a_lo", [n_tiles, D, TILE], FP8, kind="ExternalInput").ap()
    a_b = nc.dram_tensor("a_b", [n_tiles, 2, TILE], FP16, kind="ExternalInput").ap()
    b_hi = nc.dram_tensor(
        "b_hi", [n_tiles, 4, 128, D], FP16, kind="ExternalInput"
    ).ap()
    xw16 = nc.dram_tensor("xw16", [KC, 128, 64], FP16, kind="ExternalInput").ap()
    xw8 = nc.dram_tensor("xw8", [KC, 128, 64], FP8, kind="ExternalInput").ap()
    xwb = nc.dram_tensor("xwb", [2, 64], FP16, kind="ExternalInput").ap()
    ident = nc.dram_tensor("ident", [32, 32], F32, kind="ExternalInput").ap()

    w_out = nc.dram_tensor("w_out", [B, D], F32, kind="ExternalOutput").ap()
    m_out = nc.dram_tensor("m_out", [B, 1], F32, kind="ExternalOutput").ap()
    s_out = nc.dram_tensor("s_out", [B, 1], F32, kind="ExternalOutput").ap()

    with tile.TileContext(nc) as tc, contextlib.ExitStack() as st:
        const = st.enter_context(tc.tile_pool(name="const", bufs=1))
        apool = st.enter_context(tc.tile_pool(name="apool", bufs=2))
        bpool = st.enter_context(tc.tile_pool(name="bpool", bufs=2))
        small = st.enter_context(tc.tile_pool(name="small", bufs=3))
        pwork = st.enter_context(tc.tile_pool(name="pwork", bufs=2))
        ps_cross = st.enter_context(tc.tile_pool(name="ps_cross", bufs=1, space="PSUM"))
        ps_pt = st.enter_context(tc.tile_pool(name="ps_pt", bufs=1, space="PSUM"))
        ps_w = st.enter_context(tc.tile_pool(name="ps_w", bufs=1, space="PSUM"))

        xw16_s = const.tile([128, KC, 64], FP16)
        nc.sync.dma_start(xw16_s[:], xw16.rearrange("k p j -> p k j"))
        xw8_s = const.tile([128, KC, 64], FP8)
        nc.sync.dma_start(xw8_s[:], xw8.rearrange("k p j -> p k j"))
        xwb_s = const.tile([2, 64], FP16)
        nc.sync.dma_start(xwb_s[:], xwb)
        ident_s = const.tile([32, 32], F32)
        nc.sync.dma_start(ident_s[:], ident)

        W_acc = const.tile([B, D], F32)
        nc.vector.memset(W_acc[:], 0.0)
        m_run = const.tile([B, 1], F32)
        nc.vector.memset(m_run[:], -1e30)
        s_run = const.tile([B, 1], F32)
        nc.vector.memset(s_run[:], 0.0)

        for i in [t for _ in range(repeat) for t in range(n_tiles)]:
            a16_t = apool.tile([128, KC, TILE], FP16, tag="a16")
            a8_t = apool.tile([128, KC, TILE], FP8, tag="a8")
            ab_t = apool.tile([2, TILE], FP16, tag="ab")
            b16_t = bpool.tile([128, 4, D], FP16, tag="b16")
            if not skip_dma:
                nc.sync.dma_start(
                    a16_t[:], a_hi[i].rearrange("(k p) n -> p k n", p=128)
                )
                nc.sync.dma_start(
                    a8_t[:], a_lo[i].rearrange("(k p) n -> p k n", p=128)
                )
                nc.sync.dma_start(ab_t[:], a_b[i])
                nc.sync.dma_start(b16_t[:], b_hi[i].rearrange("c p d -> p c d"))
            if skip_compute:
                # consume each tile so DCE keeps the DMAs
                dmy = small.tile([128, 1], F32, tag="dmy")
                nc.vector.reduce_max(dmy[:], a16_t[:, 0, 0:8], axis=mybir.AxisListType.X)
                nc.vector.reduce_max(dmy[:], a8_t[:, 0, 0:8], axis=mybir.AxisListType.X)
                nc.vector.reduce_max(dmy[0:2, :], ab_t[:, 0:8], axis=mybir.AxisListType.X)
                nc.vector.reduce_max(dmy[:], b16_t[:, 0, 0:8], axis=mybir.AxisListType.X)
                continue

            cross = ps_cross.tile([64, TILE], F32, tag="cross")
            for k in range(KC):
                nc.tensor.matmul(
                    cross[:], xw16_s[:, k], a16_t[:, k], start=(k == 0), stop=False
                )
            nc.tensor.matmul(cross[:], xwb_s[:], ab_t[:], start=False, stop=False)
            for k in range(KC):
                nc.tensor.matmul(
                    cross[:], xw8_s[:, k], a8_t[:, k],
                    start=False, stop=(k == KC - 1),
                )

            # logits = cross[x_hi rows] + cross[x_lo rows]; one PSUM read per op
            crossb = pwork.tile([B, TILE], F32, tag="crossb")
            nc.scalar.copy(crossb[:], cross[B:64, :])
            logits = pwork.tile([B, TILE], F32, tag="logits")
            nc.vector.tensor_add(logits[:], cross[0:B, :], crossb[:])

            mt = small.tile([B, 1], F32, tag="mt")
            nc.vector.reduce_max(mt[:], logits[:], axis=mybir.AxisListType.X)
            mnew = small.tile([B, 1], F32, tag="mnew")
            nc.vector.tensor_max(mnew[:], mt[:], m_run[:])
            dm = small.tile([B, 1], F32, tag="dm")
            nc.vector.tensor_sub(dm[:], m_run[:], mnew[:])
            fsc = small.tile([B, 1], F32, tag="fsc")
            nc.scalar.activation(fsc[:], dm[:], mybir.ActivationFunctionType.Exp)
            nc.vector.tensor_copy(m_run[:], mnew[:])
            negm = small.tile([B, 1], F32, tag="negm")
            nc.vector.tensor_scalar_mul(negm[:], mnew[:], -1.0)

            p = pwork.tile([B, TILE], F32, tag="p")
            rowsum = small.tile([B, 1], F32, tag="rowsum")
            nc.scalar.activation(
                p[:], logits[:], mybir.ActivationFunctionType.Exp,
                bias=negm[:], scale=1.0, accum_out=rowsum[:],
            )
            nc.vector.scalar_tensor_tensor(
                s_run[:], s_run[:], fsc[:], rowsum[:],
                mybir.AluOpType.mult, mybir.AluOpType.add,
            )

            pTp = ps_pt.tile([128, 128], F32, tag="pT")
            for cb in range(4):
                nc.tensor.transpose(
                    pTp[:, cb * 32 : (cb + 1) * 32],
                    p[:, cb * 128 : (cb + 1) * 128],
                    ident_s[:],
                )
            pT16 = pwork.tile([128, 128], FP16, tag="pT16")
            nc.vector.tensor_copy(pT16[:], pTp[:])

            wp = ps_w.tile([B, D], F32, tag="wp")
            for cb in range(4):
                for jb in range(D // 512):
                    sl = slice(jb * 512, (jb + 1) * 512)
                    nc.tensor.matmul(
                        wp[:, sl],
                        pT16[:, cb * 32 : (cb + 1) * 32],
                        b16_t[:, cb, sl],
                        start=(cb == 0),
                        stop=(cb == 3),
                    )
            nc.vector.scalar_tensor_tensor(
                W_acc[:], W_acc[:], fsc[:], wp[:],
                mybir.AluOpType.mult, mybir.AluOpType.add,
            )

        nc.sync.dma_start(w_out, W_acc[:])
        nc.sync.dma_start(m_out, m_run[:])
        nc.sync.dma_start(s_out, s_run[:])

    nc.compile()
    return nc


_NC_CACHE = {}


def _get_nc(n_tiles=N_TILES):
    if n_tiles not in _NC_CACHE:
        _NC_CACHE[n_tiles] = build_nc(n_tiles)
    return _NC_CACHE[n_tiles]


LAST_RESULT = None  # BassKernelResults of the most recent run (for test harness)
LAST_IN_MAPS = None  # per-core input dicts of the most recent run


def kernel(x, train_data, alphas_cumprod, t):
    x = np.asarray(x)
    train_data = np.asarray(train_data)
    alphas_cumprod = np.asarray(alphas_cumprod)
    t_idx = int(np.asarray(t))

    ab = float(alphas_cumprod[t_idx])
    s_ab = np.sqrt(ab)
    one_minus = 1.0 - ab
    coefA = s_ab / one_minus            # logits = coefA * (x . t) - coefB * t_sq
    coefB = ab / (2.0 * one_minus)
    inv = 1.0 / np.sqrt(one_minus)

    xf = x.reshape(B, D).astype(np.float64)
    xs = coefA * xf                      # fold coefA into the query side

    # x-side stationary operands (shared across cores)
    xs_hi = xs.astype(np.float16)
    xs_lo = (xs - xs_hi.astype(np.float64)).astype(np.float16)
    x8 = xs / LO_SCALE
    x8_hi = x8.astype(NP_FP8)
    x8_lo = (x8 - x8_hi.astype(np.float64)).astype(NP_FP8)
    xw16 = np.zeros((KC, 128, 64), np.float16)
    xw8 = np.zeros((KC, 128, 64), NP_FP8)
    for k in range(KC):
        sl = slice(k * 128, (k + 1) * 128)
        xw16[k, :, 0:B] = xs_hi[:, sl].T
        xw16[k, :, B:64] = xs_lo[:, sl].T
        xw8[k, :, 0:B] = x8_hi[:, sl].T
        xw8[k, :, B:64] = x8_lo[:, sl].T
    xwb = np.zeros((2, 64), np.float16)
    xwb[0, 0:B] = 1.0
    xwb[1, 0:B] = 1.0
    ident = np.eye(32, dtype=np.float32)

    tf = train_data.reshape(N, D)
    in_maps = []
    for c in range(N_CORES):
        shard = tf[c * N_SHARD : (c + 1) * N_SHARD].astype(np.float32)
        t_pad = np.zeros((N_PAD, D), np.float32)
        t_pad[:N_SHARD] = shard

        td = t_pad.astype(np.float64)
        t_sq = np.einsum("nd,nd->n", td, td)
        bias = -coefB * (t_sq - float(D))
        bias[N_SHARD:] = PAD_BIAS

        At = td.T                                    # [D, N_PAD]
        A_hi16 = At.astype(np.float16)
        A_lo8 = ((At - A_hi16.astype(np.float64)) * LO_SCALE).astype(NP_FP8)
        a_hi = np.ascontiguousarray(
            A_hi16.reshape(D, N_TILES, TILE).transpose(1, 0, 2)
        )
        a_lo = np.ascontiguousarray(
            A_lo8.reshape(D, N_TILES, TILE).transpose(1, 0, 2)
        )

        bias_hi = bias.astype(np.float16)
        bias_lo = (bias - bias_hi.astype(np.float64)).astype(np.float16)
        a_b = np.ascontiguousarray(
            np.stack([bias_hi, bias_lo])             # [2, N_PAD]
            .reshape(2, N_TILES, TILE)
            .transpose(1, 0, 2)
        ).astype(np.float16)

        b_hi = t_pad.astype(np.float16).reshape(N_TILES, 4, 128, D)

        in_maps.append(
            dict(
                a_hi=a_hi, a_lo=a_lo, a_b=a_b, b_hi=b_hi,
                xw16=xw16, xw8=xw8, xwb=xwb, ident=ident,
            )
        )

    nc = _get_nc()
    res = bass_utils.run_bass_kernel_spmd(nc, in_maps, core_ids=list(range(N_CORES)))
    global LAST_RESULT, LAST_IN_MAPS
    LAST_RESULT = res
    LAST_IN_MAPS = in_maps

    Wc = np.stack([r["w_out"] for r in res.results]).astype(np.float64)  # [8,B,D]
    mc = np.stack([r["m_out"][:, 0] for r in res.results]).astype(np.float64)
    sc = np.stack([r["s_out"][:, 0] for r in res.results]).astype(np.float64)

    M = mc.max(0)                                    # [B]
    fac = np.exp(mc - M[None, :])                    # [8, B]
    W_tot = np.einsum("cb,cbd->bd", fac, Wc)
    s_tot = (fac * sc).sum(0)                        # [B]
    weighted = W_tot / s_tot[:, None]                # [B, D]

    out = inv * xf - (s_ab * inv) * weighted
    return out.reshape(x.shape).astype(np.float32)


# revision 11
# speedup vs baseline: 1.3250x; 1.1259x over previous
"""Trainium2 Bass kernel for nn_DiffusionStar (retrieval_knn).

Computes eps_star = (x - sqrt(ab) * weighted_x) / sqrt(1 - ab) where
weighted_x is the softmax-weighted average of the train set under the
Gaussian kernel exp(-||x - sqrt(ab) x0||^2 / (2 (1 - ab))).

Strategy (per sharding hint): shard train_data along N across 8 cores.
Each core streams its shard once, computing per-tile logits via a PE
matmul (contraction over D=3072 needs a D-major operand, so the host
uploads the shard transposed, split into fp16-hi + scaled-fp8-lo so the
logit precision matches f32), tracks a running row max, exponentiates
with the running max as bias (online softmax), and accumulates the
weighted sum W = p @ train via a second PE matmul off the natively-laid
fp16 copy, rescaling W by exp(m_old - m_new) each tile. Cores return
partial (W, m, s); the host merges with the standard online-softmax
combine and applies the final elementwise formula.
"""

import contextlib

import ml_dtypes
import numpy as np

from concourse import bacc, bass, mybir, tile
from concourse import bass_utils

FP16 = mybir.dt.float16
FP8 = mybir.dt.float8e4
F32 = mybir.dt.float32
NP_FP8 = ml_dtypes.float8_e4m3

B = 32          # queries
D = 3072        # feature dim (c*h*w)
N = 100000      # train points
N_CORES = 8
N_SHARD = N // N_CORES          # 12500
TILE = 512
N_TILES = (N_SHARD + TILE - 1) // TILE   # 25
N_PAD = N_TILES * TILE                   # 12800
KC = D // 128                            # 24 contraction chunks
LO_SCALE = 64.0                          # fp8 lo-residual scale
PAD_BIAS = -30000.0                      # logit bias for padded rows


def build_nc(n_tiles=N_TILES, repeat=1, skip_compute=False, skip_dma=False):
    nc = bacc.Bacc("TRN2", target_bir_lowering=False, debug=False, num_devices=1)

    a_hi = nc.dram_tensor("a_hi", [n_tiles, D, TILE], FP16, kind="ExternalInput").ap()
    a_lo = nc.dram_tensor("a_lo", [n_tiles, D, TILE], FP8, kind="ExternalInput").ap()
    a_b = nc.dram_tensor("a_b", [n_tiles, 2, TILE], FP16, kind="ExternalInput").ap()
    b_hi = nc.dram_tensor(
        "b_hi", [n_tiles, 4, 128, D], FP16, kind="ExternalInput"
    ).ap()
    xw16 = nc.dram_tensor("xw16", [KC, 128, 64], FP16, kind="ExternalInput").ap()
    xw8 = nc.dram_tensor("xw8", [KC, 128, 64], FP8, kind="ExternalInput").ap()
    xwb = nc.dram_tensor("xwb", [2, 64], FP16, kind="ExternalInput").ap()
    ident = nc.dram_tensor("ident", [32, 32], F32, kind="ExternalInput").ap()

    w_out = nc.dram_tensor("w_out", [B, D], F32, kind="ExternalOutput").ap()
    m_out = nc.dram_tensor("m_out", [B, 1], F32, kind="ExternalOutput").ap()
    s_out = nc.dram_tensor("s_out", [B, 1], F32, kind="ExternalOutput").ap()

    with tile.TileContext(nc) as tc, contextlib.ExitStack() as st:
        const = st.enter_context(tc.tile_pool(name="const", bufs=1))
        apool = st.enter_context(tc.tile_pool(name="apool", bufs=2))
        bpool = st.enter_context(tc.tile_pool(name="bpool", bufs=2))
        small = st.enter_context(tc.tile_pool(name="small", bufs=3))
        pwork = st.enter_context(tc.tile_pool(name="pwork", bufs=2))
        ps_cross = st.enter_context(tc.tile_pool(name="ps_cross", bufs=1, space="PSUM"))
        ps_pt = st.enter_context(tc.tile_pool(name="ps_pt", bufs=1, space="PSUM"))
        ps_w = st.enter_context(tc.tile_pool(name="ps_w", bufs=1, space="PSUM"))

        xw16_s = const.tile([128, KC, 64], FP16)
        nc.sync.dma_start(xw16_s[:], xw16.rearrange("k p j -> p k j"))
        xw8_s = const.tile([128, KC, 64], FP8)
        nc.sync.dma_start(xw8_s[:], xw8.rearrange("k p j -> p k j"))
        xwb_s = const.tile([2, 64], FP16)
        nc.sync.dma_start(xwb_s[:], xwb)
        ident_s = const.tile([32, 32], F32)
        nc.sync.dma_start(ident_s[:], ident)

        W_acc = const.tile([B, D], F32)
        nc.vector.memset(W_acc[:], 0.0)
        m_run = const.tile([B, 1], F32)
        nc.vector.memset(m_run[:], -1e30)
        s_run = const.tile([B, 1], F32)
        nc.vector.memset(s_run[:], 0.0)

        for i in [t for _ in range(repeat) for t in range(n_tiles)]:
            a16_t = apool.tile([128, KC, TILE], FP16, tag="a16")
            a8_t = apool.tile([128, KC, TILE], FP8, tag="a8")
            ab_t = apool.tile([2, TILE], FP16, tag="ab")
            b16_t = bpool.tile([128, 4, D], FP16, tag="b16")
            if not skip_dma:
                nc.sync.dma_start(
                    a16_t[:], a_hi[i].rearrange("(k p) n -> p k n", p=128)
                )
                nc.sync.dma_start(
                    a8_t[:], a_lo[i].rearrange("(k p) n -> p k n", p=128)
                )
                nc.sync.dma_start(ab_t[:], a_b[i])
                nc.sync.dma_start(b16_t[:], b_hi[i].rearrange("c p d -> p c d"))
            if skip_dma:
                # touch tiles so the Tile allocator sees a writer
                nc.vector.memset(a16_t[:, 0, 0:2], 0.0)
                nc.vector.memset(a8_t[:, 0, 0:2], 0.0)
                nc.vector.memset(ab_t[:, 0:2], 0.0)
                nc.vector.memset(b16_t[:, 0, 0:2], 0.0)
            if skip_compute:
                # consume each tile so DCE keeps the DMAs
                dmy = small.tile([128, 1], F32, tag="dmy")
                nc.vector.reduce_max(dmy[:], a16_t[:, 0, 0:8], axis=mybir.AxisListType.X)
                nc.vector.reduce_max(dmy[:], a8_t[:, 0, 0:8], axis=mybir.AxisListType.X)
                nc.vector.reduce_max(dmy[0:2, :], ab_t[:, 0:8], axis=mybir.AxisListType.X)
                nc.vector.reduce_max(dmy[:], b16_t[:, 0, 0:8], axis=mybir.AxisListType.X)
                continue

            cross = ps_cross.tile([64, TILE], F32, tag="cross")
            for k in range(KC):
                nc.tensor.matmul(
                    cross[:], xw16_s[:, k], a16_t[:, k], start=(k == 0), stop=False
                )
            nc.tensor.matmul(cross[:], xwb_s[:], ab_t[:], start=False, stop=False)
            for k in range(KC):
                nc.tensor.matmul(
                    cross[:], xw8_s[:, k], a8_t[:, k],
                    start=False, stop=(k == KC - 1),
                )

            # logits = cross[x_hi rows] + cross[x_lo rows]; one PSUM read per op
            crossb = pwork.tile([B, TILE], F32, tag="crossb")
            nc.scalar.copy(crossb[:], cross[B:64, :])
            logits = pwork.tile([B, TILE], F32, tag="logits")
            nc.vector.tensor_add(logits[:], cross[0:B, :], crossb[:])

            mt = small.tile([B, 1], F32, tag="mt")
            nc.vector.reduce_max(mt[:], logits[:], axis=mybir.AxisListType.X)
            mnew = small.tile([B, 1], F32, tag="mnew")
            nc.vector.tensor_max(mnew[:], mt[:], m_run[:])
            dm = small.tile([B, 1], F32, tag="dm")
            nc.vector.tensor_sub(dm[:], m_run[:], mnew[:])
            fsc = small.tile([B, 1], F32, tag="fsc")
            nc.scalar.activation(fsc[:], dm[:], mybir.ActivationFunctionType.Exp)
            nc.vector.tensor_copy(m_run[:], mnew[:])
            negm = small.tile([B, 1], F32, tag="negm")
            nc.vector.tensor_scalar_mul(negm[:], mnew[:], -1.0)

            p = pwork.tile([B, TILE], F32, tag="p")
            rowsum = small.tile([B, 1], F32, tag="rowsum")
            nc.scalar.activation(
                p[:], logits[:], mybir.ActivationFunctionType.Exp,
                bias=negm[:], scale=1.0, accum_out=rowsum[:],
            )
            nc.vector.scalar_tensor_tensor(
                s_run[:], s_run[:], fsc[:], rowsum[:],
                mybir.AluOpType.mult, mybir.AluOpType.add,
            )

            pTp = ps_pt.tile([128, 128], F32, tag="pT")
            for cb in range(4):
                nc.tensor.transpose(
                    pTp[:, cb * 32 : (cb + 1) * 32],
                    p[:, cb * 128 : (cb + 1) * 128],
                    ident_s[:],
                )
            pT16 = pwork.tile([128, 128], FP16, tag="pT16")
            nc.vector.tensor_copy(pT16[:], pTp[:])

            wp = ps_w.tile([B, D], F32, tag="wp")
            for cb in range(4):
                for jb in range(D // 512):
                    sl = slice(jb * 512, (jb + 1) * 512)
                    nc.tensor.matmul(
                        wp[:, sl],
                        pT16[:, cb * 32 : (cb + 1) * 32],
                        b16_t[:, cb, sl],
                        start=(cb == 0),
                        stop=(cb == 3),
                    )
            nc.vector.scalar_tensor_tensor(
                W_acc[:], W_acc[:], fsc[:], wp[:],
                mybir.AluOpType.mult, mybir.AluOpType.add,
            )

        nc.sync.dma_start(w_out, W_acc[:])
        nc.sync.dma_start(m_out, m_run[:])
        nc.sync.dma_start(s_out, s_run[:])

    nc.compile()
    return nc


_NC_CACHE = {}


def _get_nc(n_tiles=N_TILES):
    if n_tiles not in _NC_CACHE:
        _NC_CACHE[n_tiles] = build_nc(n_tiles)
    return _NC_CACHE[n_tiles]


LAST_RESULT = None  # BassKernelResults of the most recent run (for test harness)
LAST_IN_MAPS = None  # per-core input dicts of the most recent run


def kernel(x, train_data, alphas_cumprod, t):
    x = np.asarray(x)
    train_data = np.asarray(train_data)
    alphas_cumprod = np.asarray(alphas_cumprod)
    t_idx = int(np.asarray(t))

    ab = float(alphas_cumprod[t_idx])
    s_ab = np.sqrt(ab)
    one_minus = 1.0 - ab
    coefA = s_ab / one_minus            # logits = coefA * (x . t) - coefB * t_sq
    coefB = ab / (2.0 * one_minus)
    inv = 1.0 / np.sqrt(one_minus)

    xf = x.reshape(B, D).astype(np.float64)
    xs = coefA * xf                      # fold coefA into the query side

    # x-side stationary operands (shared across cores)
    xs_hi = xs.astype(np.float16)
    xs_lo = (xs - xs_hi.astype(np.float64)).astype(np.float16)
    x8 = xs / LO_SCALE
    x8_hi = x8.astype(NP_FP8)
    x8_lo = (x8 - x8_hi.astype(np.float64)).astype(NP_FP8)
    xw16 = np.zeros((KC, 128, 64), np.float16)
    xw8 = np.zeros((KC, 128, 64), NP_FP8)
    for k in range(KC):
        sl = slice(k * 128, (k + 1) * 128)
        xw16[k, :, 0:B] = xs_hi[:, sl].T
        xw16[k, :, B:64] = xs_lo[:, sl].T
        xw8[k, :, 0:B] = x8_hi[:, sl].T
        xw8[k, :, B:64] = x8_lo[:, sl].T
    xwb = np.zeros((2, 64), np.float16)
    xwb[0, 0:B] = 1.0
    xwb[1, 0:B] = 1.0
    ident = np.eye(32, dtype=np.float32)

    tf = train_data.reshape(N, D)
    in_maps = []
    for c in range(N_CORES):
        shard = tf[c * N_SHARD : (c + 1) * N_SHARD].astype(np.float32)
        t_pad = np.zeros((N_PAD, D), np.float32)
        t_pad[:N_SHARD] = shard

        td = t_pad.astype(np.float64)
        t_sq = np.einsum("nd,nd->n", td, td)
        bias = -coefB * (t_sq - float(D))
        bias[N_SHARD:] = PAD_BIAS

        At = td.T                                    # [D, N_PAD]
        A_hi16 = At.astype(np.float16)
        A_lo8 = ((At - A_hi16.astype(np.float64)) * LO_SCALE).astype(NP_FP8)
        a_hi = np.ascontiguousarray(
            A_hi16.reshape(D, N_TILES, TILE).transpose(1, 0, 2)
        )
        a_lo = np.ascontiguousarray(
            A_lo8.reshape(D, N_TILES, TILE).transpose(1, 0, 2)
        )

        bias_hi = bias.astype(np.float16)
        bias_lo = (bias - bias_hi.astype(np.float64)).astype(np.float16)
        a_b = np.ascontiguousarray(
            np.stack([bias_hi, bias_lo])             # [2, N_PAD]
            .reshape(2, N_TILES, TILE)
            .transpose(1, 0, 2)
        ).astype(np.float16)

        b_hi = t_pad.astype(np.float16).reshape(N_TILES, 4, 128, D)

        in_maps.append(
            dict(
                a_hi=a_hi, a_lo=a_lo, a_b=a_b, b_hi=b_hi,
                xw16=xw16, xw8=xw8, xwb=xwb, ident=ident,
            )
        )

    nc = _get_nc()
    res = bass_utils.run_bass_kernel_spmd(nc, in_maps, core_ids=list(range(N_CORES)))
    global LAST_RESULT, LAST_IN_MAPS
    LAST_RESULT = res
    LAST_IN_MAPS = in_maps

    Wc = np.stack([r["w_out"] for r in res.results]).astype(np.float64)  # [8,B,D]
    mc = np.stack([r["m_out"][:, 0] for r in res.results]).astype(np.float64)
    sc = np.stack([r["s_out"][:, 0] for r in res.results]).astype(np.float64)

    M = mc.max(0)                                    # [B]
    fac = np.exp(mc - M[None, :])                    # [8, B]
    W_tot = np.einsum("cb,cbd->bd", fac, Wc)
    s_tot = (fac * sc).sum(0)                        # [B]
    weighted = W_tot / s_tot[:, None]                # [B, D]

    out = inv * xf - (s_ab * inv) * weighted
    return out.reshape(x.shape).astype(np.float32)
